# revision 1
# baseline (speedup 1.0000x reference)
"""Causal dot-product attention (B=4, H=16, S=2048, D=128) on 8 TRN2 NeuronCores.

Sharding: batch*heads = 64 (b,h) pairs -> 8 heads per core (head parallel, no
communication). Each core runs a flash-attention-style kernel:

  - Host pre-transposes Q,K per head to [D=128, S] (fp32) so both matmul
    operands have the contraction dim (D) on partitions, and packs V per head
    as [kpos=128, kblock, D+1] in bf16 with a ones column at d=128 (the PV
    matmul then produces the softmax denominator for free).
  - Device computes S^T blocks st[k, q] = K^T.T @ Q^T via float32r matmuls
    (moving dim 256 -> full PE rate), exp(scale*st) on the scalar engine
    (PSUM -> bf16 SBUF), a triangular-mask multiply on diagonal blocks only
    (DVE), then PV: out[q, 0:129] += pt_block.T @ V_aug in bf16, accumulated
    over k-blocks in PSUM. Block-causal skipping halves the work.
  - Normalize: out = acc[:, :128] * reciprocal(acc[:, 128]) on DVE, DMA out.

No max-subtraction is needed: scores are ~N(0,1) after the 1/sqrt(128) scale,
so exp() stays in [e-6, e+6] comfortably inside fp32/bf16 range.
"""

import math
import sys
from contextlib import ExitStack

import numpy as np

for _p in ("/opt/trn_rl_repo", "/root/.axon_site/_ro/trn_rl_repo"):
    if _p not in sys.path:
        sys.path.append(_p)

import ml_dtypes

import concourse.bass as bass
import concourse.tile as tile
from concourse import bacc, mybir
from concourse.bass_utils import run_bass_kernel_spmd

F32 = mybir.dt.float32
F32R = mybir.dt.float32r
BF16 = mybir.dt.bfloat16
AF = mybir.ActivationFunctionType

# Problem constants (hardcoded; kernel.py must be self-contained).
B, H, S, D = 4, 16, 2048, 128
P = 128
N_CORES = 8
NH = (B * H) // N_CORES  # heads per core = 8
SCALE = 1.0 / math.sqrt(128.0)  # D_MODEL = 128

QTW = 256  # q-tile width (matmul moving dim; >=256 keeps float32r at full rate)
GROUP = 4  # k-chunks per PSUM st tile (2 banks) / per exp() instruction


def build_nc(nh=NH, s=S, qk_dtype=F32R, pv_dtype=BF16):
    nkb = s // P  # k-blocks per head
    nqt = s // QTW  # q-tiles per head
    sub = QTW // P  # q-subtiles (of 128) per q-tile

    nc = bacc.Bacc("TRN2", target_bir_lowering=False, debug=False,
                   enable_asserts=False)
    qt_d = nc.declare_dram_parameter("qt", [nh, P, s], qk_dtype,
                                     isOutput=False).ap()
    kt_d = nc.declare_dram_parameter("kt", [nh, P, s], qk_dtype,
                                     isOutput=False).ap()
    v_d = nc.declare_dram_parameter("v", [nh, P, nkb, D + 1], BF16,
                                    isOutput=False).ap()
    mask_d = nc.declare_dram_parameter("mask", [P, P], BF16, isOutput=False).ap()
    out_d = nc.declare_dram_parameter("out", [nh, s, D], F32, isOutput=True).ap()

    with tile.TileContext(nc) as tc, ExitStack() as ctx:
        kt_pool = ctx.enter_context(tc.tile_pool(name="kt_pool", bufs=2))
        v_pool = ctx.enter_context(tc.tile_pool(name="v_pool", bufs=2))
        qt_pool = ctx.enter_context(tc.tile_pool(name="qt_pool", bufs=5))
        pt_pool = ctx.enter_context(tc.tile_pool(name="pt_pool", bufs=6))
        st_pool = ctx.enter_context(tc.tile_pool(name="st_pool", bufs=2,
                                                 space="PSUM"))
        acc_pool = ctx.enter_context(tc.tile_pool(name="acc_pool", bufs=4,
                                                  space="PSUM"))
        out_pool = ctx.enter_context(tc.tile_pool(name="out_pool", bufs=8))
        rl_pool = ctx.enter_context(tc.tile_pool(name="rl_pool", bufs=8))
        misc = ctx.enter_context(tc.tile_pool(name="misc", bufs=1))

        mask_t = misc.tile([P, P], BF16)
        nc.sync.dma_start(out=mask_t[:], in_=mask_d)

        # Streaming state: st/pt tiles fill with up to GROUP k-chunks before a
        # single exp() drains them; the stream runs across q-tile boundaries.
        # PV consumption of a group is deferred TWO groups: when PV(g) reaches
        # the PE queue head, its dependency exp(g) finished two ACT-periods
        # ago, so the in-order PE queue never head-of-line blocks ready QK
        # work behind a PV that waits on the in-flight exp.
        state = {"st": None, "pt": None, "fill": 0, "entries": [],
                 "pending": []}

        def normalize(h, i, acc_t):
            for sI in range(sub):
                g = i * sub + sI
                rl = rl_pool.tile([P, 1], F32, tag="rl", name="rl")
                nc.vector.reciprocal(rl[:], acc_t[:, sI * 129 + 128:sI * 129 + 129])
                o_t = out_pool.tile([P, D], F32, tag="o", name="o_t")
                nc.vector.tensor_scalar_mul(o_t[:], acc_t[:, sI * 129:sI * 129 + D],
                                            rl[:])
                # Output stores go on the (otherwise idle) GpSimd queue so
                # their normalize-waits never head-of-line block the sync
                # queue that prefetches the next head's K/V/Q.
                nc.gpsimd.dma_start(out=out_d[h, g * P:(g + 1) * P, :], in_=o_t[:])

        def emit_pv(group):
            pt_t, entries = group
            for (pos, eh, i, j, acc_t, v_t) in entries:
                base = pos
                for sI in range(sub):
                    g = i * sub + sI  # global q-block index
                    if j > g:
                        continue  # fully-masked block: skip PV entirely
                    ps = pt_t[:, base + sI * P: base + (sI + 1) * P]
                    if j == g:
                        nc.vector.tensor_mul(ps, ps, mask_t[:])
                    # One PSUM accumulation group per acc bank: start=True arms
                    # the whole 2KB zero region, so only the first matmul into
                    # the tile starts and only the last one stops.
                    nc.tensor.matmul(acc_t[:, sI * 129:(sI + 1) * 129],
                                     lhsT=ps, rhs=v_t[:, j],
                                     start=(j == 0 and sI == 0),
                                     stop=(sI == sub - 1 and j == i * sub + sub - 1))
            for (pos, eh, i, j, acc_t, v_t) in entries:
                if j == (i + 1) * sub - 1:
                    normalize(eh, i, acc_t)

        def flush(final=False):
            pend = state["pending"]
            if state["fill"]:
                w = state["fill"]  # fill is in columns
                st_t, pt_t = state["st"], state["pt"]
                nc.scalar.activation(pt_t[:, :w], st_t[:, :w], AF.Exp,
                                     bias=0.0, scale=SCALE)
                pend.append((pt_t, state["entries"]))
            lag = 0 if final else 2
            while len(pend) > lag:
                emit_pv(pend.pop(0))
            state.update(st=None, pt=None, fill=0, entries=[], pending=pend)

        PRE = min(512, s)  # kt cols prefetched a head ahead
        stash = {}

        def start_head(h, pre_only):
            """Allocate head h's kt/v tiles and emit (part of) their loads.

            pre_only=True: only the first PRE kt cols + first v chunk (called
            from late in head h-1 so head h's first groups never wait on DMA).
            pre_only=False: the remaining chunks.
            """
            vchunk = max(1, nkb // 4)
            if pre_only or h not in stash:
                kt_t = kt_pool.tile([P, s], qk_dtype, tag="kt", name="kt_t")
                v_t = v_pool.tile([P, nkb, D + 1], pv_dtype, tag="v", name="v_t")
                stash[h] = (kt_t, v_t)
                w0 = 128 if h == 0 else 256  # finer first chunks for head 0
                for c in range(0, PRE, w0):
                    nc.sync.dma_start(out=kt_t[:, c:c + w0],
                                      in_=kt_d[h, :, c:c + w0])
                nc.sync.dma_start(out=v_t[:, :vchunk], in_=v_d[h, :, :vchunk])
                if pre_only:
                    return
            kt_t, v_t = stash[h]
            if h == 0:
                return  # head 0's bulk loads interleave with its qt stream
            for c in range(PRE, s, 256):
                nc.sync.dma_start(out=kt_t[:, c:c + 256],
                                  in_=kt_d[h, :, c:c + 256])
            for c in range(vchunk, nkb, vchunk):
                nc.sync.dma_start(out=v_t[:, c:c + vchunk],
                                  in_=v_d[h, :, c:c + vchunk])

        qt_early = {}
        vchunk0 = max(1, nkb // 4)
        h0_load = {"kt": PRE, "v": vchunk0}
        for h in range(nh):
            if h == 0:
                # Startup: the first two qt tiles go ahead of everything else.
                for i0 in range(min(2, nqt)):
                    q = qt_pool.tile([P, QTW], qk_dtype, tag="qt", name="qt_t")
                    nc.sync.dma_start(out=q[:, :QTW // 2],
                                      in_=qt_d[0, :, i0 * QTW:i0 * QTW + QTW // 2])
                    nc.sync.dma_start(out=q[:, QTW // 2:],
                                      in_=qt_d[0, :, i0 * QTW + QTW // 2:(i0 + 1) * QTW])
                    qt_early[i0] = q
                start_head(0, pre_only=True)
            start_head(h, pre_only=False)
            kt_t, v_t = stash[h]

            for i in range(nqt):
                if h == 0 and i in qt_early:
                    qt_t = qt_early[i]
                else:
                    qt_t = qt_pool.tile([P, QTW], qk_dtype, tag="qt", name="qt_t")
                    nc.sync.dma_start(out=qt_t[:, :QTW // 2],
                                      in_=qt_d[h, :, i * QTW:i * QTW + QTW // 2])
                    nc.sync.dma_start(out=qt_t[:, QTW // 2:],
                                      in_=qt_d[h, :, i * QTW + QTW // 2:(i + 1) * QTW])
                if h == 0 and i >= 2:
                    # Just-in-time bulk loads for head 0: the kt chunk (and
                    # every other q-tile, a v chunk) this q-tile's groups need.
                    last = i == nqt - 1
                    while h0_load["kt"] < s and (h0_load["kt"] < QTW * (i + 1)
                                                 or last):
                        c = h0_load["kt"]
                        nc.sync.dma_start(out=kt_t[:, c:c + QTW],
                                          in_=kt_d[0, :, c:c + QTW])
                        h0_load["kt"] += QTW
                    vchunk = max(1, nkb // 4)
                    while h0_load["v"] < nkb and (i % 2 == 0 or last):
                        cv = h0_load["v"]
                        nc.sync.dma_start(out=v_t[:, cv:cv + vchunk],
                                          in_=v_d[0, :, cv:cv + vchunk])
                        h0_load["v"] += vchunk
                        if not last:
                            break
                if i == nqt - 2 and h + 1 < nh:
                    start_head(h + 1, pre_only=True)
                acc_t = acc_pool.tile([P, sub * 129], F32, tag="acc", name="acc_t")
                for j in range((i + 1) * sub):  # causal k-blocks only
                    if state["fill"] == 0:
                        state["st"] = st_pool.tile([P, GROUP * QTW], F32,
                                                   tag="st", name="st_t")
                        state["pt"] = pt_pool.tile([P, GROUP * QTW], pv_dtype,
                                                   tag="pt", name="pt_t")
                    pos = state["fill"]
                    nc.tensor.matmul(state["st"][:, pos:pos + QTW],
                                     lhsT=kt_t[:, j * P:(j + 1) * P], rhs=qt_t[:],
                                     start=True, stop=True)
                    state["entries"].append((pos, h, i, j, acc_t, v_t))
                    state["fill"] += QTW
                    if state["fill"] == GROUP * QTW:
                        flush()
        flush(final=True)
    nc.compile()
    return nc


_NC = None


def _get_nc():
    global _NC
    if _NC is None:
        _NC = build_nc()
    return _NC


def prepare_in_maps(Q, K, V):
    """Shard + lay out full [B,H,S,D] inputs into per-core in_maps."""
    Qf = np.ascontiguousarray(np.asarray(Q, dtype=np.float32)).reshape(B * H, S, D)
    Kf = np.ascontiguousarray(np.asarray(K, dtype=np.float32)).reshape(B * H, S, D)
    Vf = np.ascontiguousarray(np.asarray(V, dtype=np.float32)).reshape(B * H, S, D)
    nkb = S // P
    mask = np.triu(np.ones((P, P), dtype=np.float32)).astype(ml_dtypes.bfloat16)
    in_maps = []
    for c in range(N_CORES):
        hs = slice(c * NH, (c + 1) * NH)
        qt = np.ascontiguousarray(Qf[hs].transpose(0, 2, 1))  # [NH, D, S]
        kt = np.ascontiguousarray(Kf[hs].transpose(0, 2, 1))  # [NH, D, S]
        # V: [NH, S, D] -> [NH, kblock, kpos, D] -> [NH, kpos, kblock, D]
        vv = Vf[hs].reshape(NH, nkb, P, D).transpose(0, 2, 1, 3)
        v_aug = np.ones((NH, P, nkb, D + 1), dtype=ml_dtypes.bfloat16)
        v_aug[..., :D] = vv.astype(ml_dtypes.bfloat16)
        in_maps.append({"qt": qt, "kt": kt, "v": v_aug, "mask": mask})
    return in_maps


def gather_out(results):
    out = np.concatenate([np.asarray(r["out"], dtype=np.float32)
                          for r in results], axis=0)  # [64, S, D]
    return out.reshape(B, H, S, D)


def kernel(Q, K, V):
    in_maps = prepare_in_maps(Q, K, V)
    nc = _get_nc()
    res = run_bass_kernel_spmd(nc, in_maps, core_ids=list(range(N_CORES)))
    return gather_out(res.results)



# revision 11
# speedup vs baseline: 1.1648x; 1.1648x over previous
"""Causal dot-product attention (B=4, H=16, S=2048, D=128) on 8 TRN2 NeuronCores.

Sharding: batch*heads = 64 (b,h) pairs -> 8 heads per core (head parallel, no
communication). Each core runs a flash-attention-style kernel.

V2 design (ACT-engine-bound; exp() on the Scalar engine is the bottleneck):
  - Host pre-transposes Q,K per head to [D=128, S] in bf16 (halves DMA vs
    fp32r, same PE rate, enables FWL weight loads), and packs V per head as
    [kpos=128, kblock, D+1] bf16 with a ones column (PV matmul produces the
    softmax denominator for free).
  - st[k, q] blocks via bf16 matmuls. Diagonal handling: for q-tile i
    (256 q cols), full-width chunks j=0..2i (moving 256); the last chunk
    j=2i+1 is computed at HALF width (only q sub-block 1, moving 128) since
    its first sub-block is fully masked -- saves ~8% of exp elements and QK
    cycles vs the v1 kernel.
  - exp(scale*st) on the scalar engine in streaming groups of ~1024 PSUM
    columns -> bf16 pt in SBUF. Triangular mask multiplies on diagonal
    blocks alternate between DVE and GpSimd queues.
  - PV: out[q, 0:129] += pt_block.T @ V_aug accumulated in PSUM per q-tile
    ([128, 2, 129] = one bank). Deferred two groups so the in-order PE queue
    never head-of-line blocks on an in-flight exp.
  - Normalize batched per q-tile: one reciprocal [128,2] + one broadcast
    tensor_mul [128,2,128] -> bf16 out tile, one DMA per q-tile. Host
    upcasts bf16 -> f32.
  - Whole-head bulk DMAs (2 per tensor) + full next-head prefetch; warmup
    activation at t=0 forces the exp table load off the critical path.

No max-subtraction is needed: scores are ~N(0,1) after the 1/sqrt(128)
scale, so exp() stays in [e-6, e+6] comfortably inside bf16 range.
"""

import math
import sys
from contextlib import ExitStack

import numpy as np

for _p in ("/opt/trn_rl_repo", "/root/.axon_site/_ro/trn_rl_repo"):
    if _p not in sys.path:
        sys.path.append(_p)

import ml_dtypes

import concourse.bass as bass
import concourse.tile as tile
from concourse import bacc, mybir
from concourse.bass_utils import run_bass_kernel_spmd

F32 = mybir.dt.float32
BF16 = mybir.dt.bfloat16
AF = mybir.ActivationFunctionType

# Problem constants (hardcoded; kernel.py must be self-contained).
B, H, S, D = 4, 16, 2048, 128
P = 128
N_CORES = 8
NH = (B * H) // N_CORES  # heads per core = 8
SCALE = 1.0 / math.sqrt(128.0)  # D_MODEL = 128

QTW = 256   # q-tile width (2 sub-blocks of 128)
FILL = 1536  # st/pt group size in columns (3 PSUM banks)


def build_nc(nh=NH, s=S):
    nkb = s // P    # k-blocks per head = 16
    nqt = s // QTW  # q-tiles per head = 8

    nc = bacc.Bacc("TRN2", target_bir_lowering=False, debug=False,
                   enable_asserts=False)
    qt_d = nc.declare_dram_parameter("qt", [nh, P, s], BF16, isOutput=False).ap()
    kt_d = nc.declare_dram_parameter("kt", [nh, P, s], BF16, isOutput=False).ap()
    v_d = nc.declare_dram_parameter("v", [nh, P, nkb, D + 1], BF16,
                                    isOutput=False).ap()
    mask_d = nc.declare_dram_parameter("mask", [P, P], BF16, isOutput=False).ap()
    out_d = nc.declare_dram_parameter("out", [nh, s, D], BF16, isOutput=True).ap()

    with tile.TileContext(nc) as tc, ExitStack() as ctx:
        kt_pool = ctx.enter_context(tc.tile_pool(name="kt_pool", bufs=2))
        v_pool = ctx.enter_context(tc.tile_pool(name="v_pool", bufs=2))
        qt_pool = ctx.enter_context(tc.tile_pool(name="qt_pool", bufs=2))
        pt_pool = ctx.enter_context(tc.tile_pool(name="pt_pool", bufs=6))
        st_pool = ctx.enter_context(tc.tile_pool(name="st_pool", bufs=2,
                                                 space="PSUM"))
        acc_pool = ctx.enter_context(tc.tile_pool(name="acc_pool", bufs=2,
                                                  space="PSUM"))
        out_pool = ctx.enter_context(tc.tile_pool(name="out_pool", bufs=4))
        rl_pool = ctx.enter_context(tc.tile_pool(name="rl_pool", bufs=4))
        misc = ctx.enter_context(tc.tile_pool(name="misc", bufs=1))

        # Warmup activation: forces the exp table load at t=0, off the
        # critical path (the real first exp otherwise pays ~2.7us).
        warm = misc.tile([P, 8], F32)
        nc.vector.memset(warm[:], 0.0)
        nc.scalar.activation(warm[:], warm[:], AF.Exp, bias=0.0, scale=1.0)

        mask_t = misc.tile([P, P], BF16)
        nc.sync.dma_start(out=mask_t[:], in_=mask_d)

        # Streaming group state: st/pt tiles fill with chunks until FILL
        # columns, then one exp() drains them; PV consumption is deferred
        # two groups (lag) so the in-order PE queue never blocks ready QK
        # work behind a PV whose exp is still in flight.
        #
        # Half-width (128-col) diagonal chunks are deferred to the group's
        # tail ("halves"): full 256-col chunks then always start at
        # 256-aligned offsets and half chunks at 128-aligned offsets, so no
        # matmul output ever crosses a 512-float PSUM bank boundary (which
        # is illegal and corrupts nondeterministically on HW).
        state = {"st": None, "pt": None, "fill": 0, "entries": [],
                 "pending": [], "mask_tick": 0, "halves": []}

        def normalize(h, i, acc_t):
            rl = rl_pool.tile([P, 2, 1], F32, tag="rl", name="rl")
            nc.vector.reciprocal(rl[:], acc_t[:, :, 128:129])
            o_t = out_pool.tile([P, 2, D], BF16, tag="o", name="o_t")
            nc.vector.tensor_mul(o_t[:], acc_t[:, :, 0:128],
                                 rl[:, :, 0:1].broadcast_to([P, 2, D]))
            dst = out_d[h, i * QTW:(i + 1) * QTW, :].rearrange(
                "(si q) d -> q si d", si=2)
            nc.gpsimd.dma_start(out=dst, in_=o_t[:])

        def emit_pv(group):
            pt_t, entries = group
            for (pos, width, eh, i, j, acc_t, v_t) in entries:
                if width == QTW:
                    # full chunk: j in 0..2i; sI=0 masked iff j==2i
                    ps0 = pt_t[:, pos:pos + P]
                    if j == 2 * i:
                        eng = (nc.vector if state["mask_tick"] % 2 == 0
                               else nc.gpsimd)
                        state["mask_tick"] += 1
                        eng.tensor_mul(ps0, ps0, mask_t[:])
                    # One PSUM accumulation group per acc bank: start=True
                    # arms the whole 2KB zero region, so only the first
                    # matmul into the tile starts and only the last stops.
                    nc.tensor.matmul(acc_t[:, 0, :], lhsT=ps0, rhs=v_t[:, j],
                                     start=(j == 0), stop=False)
                    ps1 = pt_t[:, pos + P:pos + QTW]
                    nc.tensor.matmul(acc_t[:, 1, :], lhsT=ps1, rhs=v_t[:, j],
                                     start=False, stop=False)
                else:
                    # half chunk: j == 2i+1, only q sub-block 1, always masked
                    ps1 = pt_t[:, pos:pos + P]
                    eng = (nc.vector if state["mask_tick"] % 2 == 0
                           else nc.gpsimd)
                    state["mask_tick"] += 1
                    eng.tensor_mul(ps1, ps1, mask_t[:])
                    nc.tensor.matmul(acc_t[:, 1, :], lhsT=ps1, rhs=v_t[:, j],
                                     start=False, stop=True)
            for (pos, width, eh, i, j, acc_t, v_t) in entries:
                if width != QTW:
                    normalize(eh, i, acc_t)

        def emit_qk(kt_t, qt_t, v_t, h, i, j, acc_t, width, qoff):
            pos = state["fill"]
            nc.tensor.matmul(state["st"][:, pos:pos + width],
                             lhsT=kt_t[:, j * P:(j + 1) * P],
                             rhs=qt_t[:, qoff:qoff + width],
                             start=True, stop=True)
            state["entries"].append((pos, width, h, i, j, acc_t, v_t))
            state["fill"] += width

        def flush(final=False):
            # Emit deferred half chunks at the (128-aligned) tail first.
            if state["halves"] and state["st"] is None:
                state["st"] = st_pool.tile([P, FILL], F32, tag="st", name="st_t")
                state["pt"] = pt_pool.tile([P, FILL], BF16, tag="pt", name="pt_t")
            for half in state["halves"]:
                emit_qk(*half)
            state["halves"] = []
            pend = state["pending"]
            if state["fill"]:
                w = state["fill"]
                st_t, pt_t = state["st"], state["pt"]
                nc.scalar.activation(pt_t[:, :w], st_t[:, :w], AF.Exp,
                                     bias=0.0, scale=SCALE)
                pend.append((pt_t, state["entries"]))
            lag = 0 if final else 2
            while len(pend) > lag:
                emit_pv(pend.pop(0))
            state.update(st=None, pt=None, fill=0, entries=[], pending=pend)

        def add_chunk(kt_t, qt_t, v_t, h, i, j, acc_t, width, qoff):
            if width == P:
                # Defer to this group's tail so full chunks stay 256-aligned.
                state["halves"].append(
                    (kt_t, qt_t, v_t, h, i, j, acc_t, width, qoff))
                return
            if state["fill"] + width + P * len(state["halves"]) > FILL:
                flush()
            if state["fill"] == 0:
                state["st"] = st_pool.tile([P, FILL], F32, tag="st", name="st_t")
                state["pt"] = pt_pool.tile([P, FILL], BF16, tag="pt", name="pt_t")
            emit_qk(kt_t, qt_t, v_t, h, i, j, acc_t, width, qoff)
            if state["fill"] + P * len(state["halves"]) >= FILL:
                flush()

        def load_head(h, tiles):
            """Allocate head h's tiles and emit its loads (ordered so the
            first q-tile's needs land first)."""
            kt_t = kt_pool.tile([P, s], BF16, tag="kt", name="kt_t")
            qt_t = qt_pool.tile([P, s], BF16, tag="qt", name="qt_t")
            v_t = v_pool.tile([P, nkb, D + 1], BF16, tag="v", name="v_t")
            tiles[h] = (kt_t, qt_t, v_t)
            if h == 0:
                nc.sync.dma_start(out=kt_t[:, :512], in_=kt_d[h, :, :512])
                nc.sync.dma_start(out=qt_t[:, :512], in_=qt_d[h, :, :512])
                nc.sync.dma_start(out=v_t[:, :4], in_=v_d[h, :, :4])
                nc.sync.dma_start(out=kt_t[:, 512:], in_=kt_d[h, :, 512:])
                nc.sync.dma_start(out=qt_t[:, 512:], in_=qt_d[h, :, 512:])
                nc.sync.dma_start(out=v_t[:, 4:], in_=v_d[h, :, 4:])
            else:
                nc.sync.dma_start(out=kt_t[:, :1024], in_=kt_d[h, :, :1024])
                nc.sync.dma_start(out=qt_t[:, :1024], in_=qt_d[h, :, :1024])
                nc.sync.dma_start(out=v_t[:, :8], in_=v_d[h, :, :8])
                nc.sync.dma_start(out=kt_t[:, 1024:], in_=kt_d[h, :, 1024:])
                nc.sync.dma_start(out=qt_t[:, 1024:], in_=qt_d[h, :, 1024:])
                nc.sync.dma_start(out=v_t[:, 8:], in_=v_d[h, :, 8:])

        tiles = {}
        load_head(0, tiles)
        for h in range(nh):
            kt_t, qt_t, v_t = tiles[h]
            for i in range(nqt):
                if h + 1 < nh and i == 4:
                    load_head(h + 1, tiles)
                acc_t = acc_pool.tile([P, 2, 129], F32, tag="acc", name="acc_t")
                qoff = i * QTW
                for j in range(2 * i + 1):
                    add_chunk(kt_t, qt_t, v_t, h, i, j, acc_t, QTW, qoff)
                # last (odd-diagonal) chunk at half width: q sub-block 1 only
                add_chunk(kt_t, qt_t, v_t, h, i, 2 * i + 1, acc_t, P, qoff + P)
        flush(final=True)
    nc.compile()
    return nc


_NC = None


def _get_nc():
    global _NC
    if _NC is None:
        _NC = build_nc()
    return _NC


def prepare_in_maps(Q, K, V):
    """Shard + lay out full [B,H,S,D] inputs into per-core in_maps."""
    Qf = np.ascontiguousarray(np.asarray(Q, dtype=np.float32)).reshape(B * H, S, D)
    Kf = np.ascontiguousarray(np.asarray(K, dtype=np.float32)).reshape(B * H, S, D)
    Vf = np.ascontiguousarray(np.asarray(V, dtype=np.float32)).reshape(B * H, S, D)
    nkb = S // P
    # mask[k, q] = 1 iff q >= k (keep lower-triangular scores)
    mask = np.triu(np.ones((P, P), dtype=np.float32)).astype(ml_dtypes.bfloat16)
    in_maps = []
    for c in range(N_CORES):
        hs = slice(c * NH, (c + 1) * NH)
        qt = np.ascontiguousarray(
            Qf[hs].transpose(0, 2, 1).astype(ml_dtypes.bfloat16))  # [NH, D, S]
        kt = np.ascontiguousarray(
            Kf[hs].transpose(0, 2, 1).astype(ml_dtypes.bfloat16))  # [NH, D, S]
        # V: [NH, S, D] -> [NH, kblock, kpos, D] -> [NH, kpos, kblock, D]
        vv = Vf[hs].reshape(NH, nkb, P, D).transpose(0, 2, 1, 3)
        v_aug = np.ones((NH, P, nkb, D + 1), dtype=ml_dtypes.bfloat16)
        v_aug[..., :D] = vv.astype(ml_dtypes.bfloat16)
        in_maps.append({"qt": qt, "kt": kt, "v": v_aug, "mask": mask})
    return in_maps


def gather_out(results):
    out = np.concatenate([np.asarray(r["out"]).astype(np.float32)
                          for r in results], axis=0)  # [64, S, D]
    return out.reshape(B, H, S, D)


def kernel(Q, K, V):
    in_maps = prepare_in_maps(Q, K, V)
    nc = _get_nc()
    res = run_bass_kernel_spmd(nc, in_maps, core_ids=list(range(N_CORES)))
    return gather_out(res.results)


# revision 13
# speedup vs baseline: 1.1980x; 1.0285x over previous
"""Causal dot-product attention (B=4, H=16, S=2048, D=128) on 8 TRN2 NeuronCores.

Sharding: batch*heads = 64 (b,h) pairs -> 8 heads per core (head parallel, no
communication). Each core runs a flash-attention-style kernel.

V2 design (ACT-engine-bound; exp() on the Scalar engine is the bottleneck):
  - Host pre-transposes Q,K per head to [D=128, S] in bf16 (halves DMA vs
    fp32r, same PE rate, enables FWL weight loads), and packs V per head as
    [kpos=128, kblock, D+1] bf16 with a ones column (PV matmul produces the
    softmax denominator for free).
  - st[k, q] blocks via bf16 matmuls. Diagonal handling: for q-tile i
    (256 q cols), full-width chunks j=0..2i (moving 256); the last chunk
    j=2i+1 is computed at HALF width (only q sub-block 1, moving 128) since
    its first sub-block is fully masked -- saves ~8% of exp elements and QK
    cycles vs the v1 kernel.
  - exp(scale*st) on the scalar engine in streaming groups of ~1024 PSUM
    columns -> bf16 pt in SBUF. Triangular mask multiplies on diagonal
    blocks alternate between DVE and GpSimd queues.
  - PV: out[q, 0:129] += pt_block.T @ V_aug accumulated in PSUM per q-tile
    ([128, 2, 129] = one bank). Deferred two groups so the in-order PE queue
    never head-of-line blocks on an in-flight exp.
  - Normalize batched per q-tile: one reciprocal [128,2] + one broadcast
    tensor_mul [128,2,128] -> bf16 out tile, one DMA per q-tile. Host
    upcasts bf16 -> f32.
  - Whole-head bulk DMAs (2 per tensor) + full next-head prefetch; warmup
    activation at t=0 forces the exp table load off the critical path.

No max-subtraction is needed: scores are ~N(0,1) after the 1/sqrt(128)
scale, so exp() stays in [e-6, e+6] comfortably inside bf16 range.
"""

import math
import sys
from contextlib import ExitStack

import numpy as np

for _p in ("/opt/trn_rl_repo", "/root/.axon_site/_ro/trn_rl_repo"):
    if _p not in sys.path:
        sys.path.append(_p)

import ml_dtypes

import concourse.bass as bass
import concourse.tile as tile
from concourse import bacc, mybir
from concourse.bass_utils import run_bass_kernel_spmd

F32 = mybir.dt.float32
BF16 = mybir.dt.bfloat16
AF = mybir.ActivationFunctionType

# Problem constants (hardcoded; kernel.py must be self-contained).
B, H, S, D = 4, 16, 2048, 128
P = 128
N_CORES = 8
NH = (B * H) // N_CORES  # heads per core = 8
SCALE = 1.0 / math.sqrt(128.0)  # D_MODEL = 128

QTW = 256   # q-tile width (2 sub-blocks of 128)
FILL = 1536  # st/pt group size in columns (3 PSUM banks)


def build_nc(nh=NH, s=S):
    nkb = s // P    # k-blocks per head = 16
    nqt = s // QTW  # q-tiles per head = 8

    nc = bacc.Bacc("TRN2", target_bir_lowering=False, debug=False,
                   enable_asserts=False)
    qt_d = nc.declare_dram_parameter("qt", [nh, P, s], BF16, isOutput=False).ap()
    kt_d = nc.declare_dram_parameter("kt", [nh, P, s], BF16, isOutput=False).ap()
    v_d = nc.declare_dram_parameter("v", [nh, P, nkb, D + 1], BF16,
                                    isOutput=False).ap()
    mask_d = nc.declare_dram_parameter("mask", [P, P], BF16, isOutput=False).ap()
    out_d = nc.declare_dram_parameter("out", [nh, s, D], BF16, isOutput=True).ap()

    with tile.TileContext(nc) as tc, ExitStack() as ctx:
        kt_pool = ctx.enter_context(tc.tile_pool(name="kt_pool", bufs=2))
        v_pool = ctx.enter_context(tc.tile_pool(name="v_pool", bufs=2))
        qt_pool = ctx.enter_context(tc.tile_pool(name="qt_pool", bufs=2))
        pt_pool = ctx.enter_context(tc.tile_pool(name="pt_pool", bufs=6))
        st_pool = ctx.enter_context(tc.tile_pool(name="st_pool", bufs=2,
                                                 space="PSUM"))
        acc_pool = ctx.enter_context(tc.tile_pool(name="acc_pool", bufs=2,
                                                  space="PSUM"))
        out_pool = ctx.enter_context(tc.tile_pool(name="out_pool", bufs=4))
        rl_pool = ctx.enter_context(tc.tile_pool(name="rl_pool", bufs=4))
        misc = ctx.enter_context(tc.tile_pool(name="misc", bufs=1))

        # Warmup activation: forces the exp table load at t=0, off the
        # critical path (the real first exp otherwise pays ~2.7us).
        warm = misc.tile([P, 8], F32)
        nc.vector.memset(warm[:], 0.0)
        nc.scalar.activation(warm[:], warm[:], AF.Exp, bias=0.0, scale=1.0)

        mask_t = misc.tile([P, P], BF16)
        nc.sync.dma_start(out=mask_t[:], in_=mask_d)

        # Streaming group state: st/pt tiles fill with chunks until FILL
        # columns, then one exp() drains them; PV consumption is deferred
        # two groups (lag) so the in-order PE queue never blocks ready QK
        # work behind a PV whose exp is still in flight.
        #
        # Half-width (128-col) diagonal chunks are deferred to the group's
        # tail ("halves"): full 256-col chunks then always start at
        # 256-aligned offsets and half chunks at 128-aligned offsets, so no
        # matmul output ever crosses a 512-float PSUM bank boundary (which
        # is illegal and corrupts nondeterministically on HW).
        state = {"st": None, "pt": None, "fill": 0, "entries": [],
                 "pending": [], "mask_tick": 0, "halves": []}

        def normalize(h, i, acc_t):
            rl = rl_pool.tile([P, 2, 1], F32, tag="rl", name="rl")
            nc.vector.reciprocal(rl[:], acc_t[:, :, 128:129])
            o_t = out_pool.tile([P, 2, D], BF16, tag="o", name="o_t")
            nc.vector.tensor_mul(o_t[:], acc_t[:, :, 0:128],
                                 rl[:, :, 0:1].broadcast_to([P, 2, D]))
            dst = out_d[h, i * QTW:(i + 1) * QTW, :].rearrange(
                "(si q) d -> q si d", si=2)
            nc.gpsimd.dma_start(out=dst, in_=o_t[:])

        def emit_pv(group):
            pt_t, entries = group
            for (pos, width, eh, i, j, acc_t, v_t) in entries:
                if width == QTW:
                    # full chunk: j in 0..2i; sI=0 masked iff j==2i
                    ps0 = pt_t[:, pos:pos + P]
                    if j == 2 * i:
                        eng = (nc.vector if state["mask_tick"] % 2 == 0
                               else nc.gpsimd)
                        state["mask_tick"] += 1
                        eng.tensor_mul(ps0, ps0, mask_t[:])
                    # One PSUM accumulation group per acc bank: start=True
                    # arms the whole 2KB zero region, so only the first
                    # matmul into the tile starts and only the last stops.
                    nc.tensor.matmul(acc_t[:, 0, :], lhsT=ps0, rhs=v_t[:, j],
                                     start=(j == 0), stop=False)
                    ps1 = pt_t[:, pos + P:pos + QTW]
                    nc.tensor.matmul(acc_t[:, 1, :], lhsT=ps1, rhs=v_t[:, j],
                                     start=False, stop=False)
                else:
                    # half chunk: j == 2i+1, only q sub-block 1, always masked
                    ps1 = pt_t[:, pos:pos + P]
                    eng = (nc.vector if state["mask_tick"] % 2 == 0
                           else nc.gpsimd)
                    state["mask_tick"] += 1
                    eng.tensor_mul(ps1, ps1, mask_t[:])
                    nc.tensor.matmul(acc_t[:, 1, :], lhsT=ps1, rhs=v_t[:, j],
                                     start=False, stop=True)
            for (pos, width, eh, i, j, acc_t, v_t) in entries:
                if width != QTW:
                    normalize(eh, i, acc_t)

        def emit_qk(kt_t, qt_t, v_t, h, i, j, acc_t, width, qoff):
            pos = state["fill"]
            nc.tensor.matmul(state["st"][:, pos:pos + width],
                             lhsT=kt_t[:, j * P:(j + 1) * P],
                             rhs=qt_t[:, qoff:qoff + width],
                             start=True, stop=True)
            state["entries"].append((pos, width, h, i, j, acc_t, v_t))
            state["fill"] += width

        def flush(final=False):
            # Emit deferred half chunks at the (128-aligned) tail first.
            if state["halves"] and state["st"] is None:
                state["st"] = st_pool.tile([P, FILL], F32, tag="st", name="st_t")
                state["pt"] = pt_pool.tile([P, FILL], BF16, tag="pt", name="pt_t")
            for half in state["halves"]:
                emit_qk(*half)
            state["halves"] = []
            pend = state["pending"]
            if state["fill"]:
                w = state["fill"]
                st_t, pt_t = state["st"], state["pt"]
                nc.scalar.activation(pt_t[:, :w], st_t[:, :w], AF.Exp,
                                     bias=0.0, scale=SCALE)
                pend.append((pt_t, state["entries"]))
            lag = 0 if final else 2
            while len(pend) > lag:
                emit_pv(pend.pop(0))
            state.update(st=None, pt=None, fill=0, entries=[], pending=pend)

        def add_chunk(kt_t, qt_t, v_t, h, i, j, acc_t, width, qoff):
            if width == P:
                # Defer to this group's tail so full chunks stay 256-aligned.
                state["halves"].append(
                    (kt_t, qt_t, v_t, h, i, j, acc_t, width, qoff))
                return
            if state["fill"] + width + P * len(state["halves"]) > FILL:
                flush()
            if state["fill"] == 0:
                state["st"] = st_pool.tile([P, FILL], F32, tag="st", name="st_t")
                state["pt"] = pt_pool.tile([P, FILL], BF16, tag="pt", name="pt_t")
            emit_qk(kt_t, qt_t, v_t, h, i, j, acc_t, width, qoff)
            if state["fill"] + P * len(state["halves"]) >= FILL:
                flush()

        def load_head(h, tiles):
            """Allocate head h's tiles and emit its loads (ordered so the
            first q-tile's needs land first)."""
            kt_t = kt_pool.tile([P, s], BF16, tag="kt", name="kt_t")
            qt_t = qt_pool.tile([P, s], BF16, tag="qt", name="qt_t")
            v_t = v_pool.tile([P, nkb, D + 1], BF16, tag="v", name="v_t")
            tiles[h] = (kt_t, qt_t, v_t)
            if h == 0:
                # Fine-grained first slices so the first q-tile's QK (and
                # with it the exp pipeline) starts as early as possible.
                nc.sync.dma_start(out=qt_t[:, :256], in_=qt_d[h, :, :256])
                nc.sync.dma_start(out=kt_t[:, :256], in_=kt_d[h, :, :256])
                nc.sync.dma_start(out=qt_t[:, 256:1024], in_=qt_d[h, :, 256:1024])
                nc.sync.dma_start(out=kt_t[:, 256:1024], in_=kt_d[h, :, 256:1024])
                nc.sync.dma_start(out=v_t[:, :4], in_=v_d[h, :, :4])
                nc.sync.dma_start(out=kt_t[:, 1024:], in_=kt_d[h, :, 1024:])
                nc.sync.dma_start(out=qt_t[:, 1024:], in_=qt_d[h, :, 1024:])
                nc.sync.dma_start(out=v_t[:, 4:], in_=v_d[h, :, 4:])
            else:
                nc.sync.dma_start(out=kt_t[:, :1024], in_=kt_d[h, :, :1024])
                nc.sync.dma_start(out=qt_t[:, :1024], in_=qt_d[h, :, :1024])
                nc.sync.dma_start(out=v_t[:, :8], in_=v_d[h, :, :8])
                nc.sync.dma_start(out=kt_t[:, 1024:], in_=kt_d[h, :, 1024:])
                nc.sync.dma_start(out=qt_t[:, 1024:], in_=qt_d[h, :, 1024:])
                nc.sync.dma_start(out=v_t[:, 8:], in_=v_d[h, :, 8:])

        # Forced flush points: small groups at the very start (exp begins
        # before the bulk DMAs land) and at the very end (the post-last-exp
        # PV/normalize drain is tiny).
        force_tile = {(0, 0), (0, 1)}
        force_chunk = {(nh - 1, nqt - 1, 2 * nqt - 4), (nh - 1, nqt - 1, 2 * nqt - 3)}

        tiles = {}
        load_head(0, tiles)
        for h in range(nh):
            kt_t, qt_t, v_t = tiles[h]
            for i in range(nqt):
                if h + 1 < nh and i == 4:
                    load_head(h + 1, tiles)
                acc_t = acc_pool.tile([P, 2, 129], F32, tag="acc", name="acc_t")
                qoff = i * QTW
                for j in range(2 * i + 1):
                    add_chunk(kt_t, qt_t, v_t, h, i, j, acc_t, QTW, qoff)
                    if (h, i, j) in force_chunk:
                        flush()
                # last (odd-diagonal) chunk at half width: q sub-block 1 only
                add_chunk(kt_t, qt_t, v_t, h, i, 2 * i + 1, acc_t, P, qoff + P)
                if (h, i) in force_tile:
                    flush()
        flush(final=True)
    nc.compile()
    return nc


_NC = None


def _get_nc():
    global _NC
    if _NC is None:
        _NC = build_nc()
    return _NC


def prepare_in_maps(Q, K, V):
    """Shard + lay out full [B,H,S,D] inputs into per-core in_maps."""
    Qf = np.ascontiguousarray(np.asarray(Q, dtype=np.float32)).reshape(B * H, S, D)
    Kf = np.ascontiguousarray(np.asarray(K, dtype=np.float32)).reshape(B * H, S, D)
    Vf = np.ascontiguousarray(np.asarray(V, dtype=np.float32)).reshape(B * H, S, D)
    nkb = S // P
    # mask[k, q] = 1 iff q >= k (keep lower-triangular scores)
    mask = np.triu(np.ones((P, P), dtype=np.float32)).astype(ml_dtypes.bfloat16)
    in_maps = []
    for c in range(N_CORES):
        hs = slice(c * NH, (c + 1) * NH)
        qt = np.ascontiguousarray(
            Qf[hs].transpose(0, 2, 1).astype(ml_dtypes.bfloat16))  # [NH, D, S]
        kt = np.ascontiguousarray(
            Kf[hs].transpose(0, 2, 1).astype(ml_dtypes.bfloat16))  # [NH, D, S]
        # V: [NH, S, D] -> [NH, kblock, kpos, D] -> [NH, kpos, kblock, D]
        vv = Vf[hs].reshape(NH, nkb, P, D).transpose(0, 2, 1, 3)
        v_aug = np.ones((NH, P, nkb, D + 1), dtype=ml_dtypes.bfloat16)
        v_aug[..., :D] = vv.astype(ml_dtypes.bfloat16)
        in_maps.append({"qt": qt, "kt": kt, "v": v_aug, "mask": mask})
    return in_maps


def gather_out(results):
    out = np.concatenate([np.asarray(r["out"]).astype(np.float32)
                          for r in results], axis=0)  # [64, S, D]
    return out.reshape(B, H, S, D)


def kernel(Q, K, V):
    in_maps = prepare_in_maps(Q, K, V)
    nc = _get_nc()
    res = run_bass_kernel_spmd(nc, in_maps, core_ids=list(range(N_CORES)))
    return gather_out(res.results)


# revision 18
# speedup vs baseline: 1.2180x; 1.0167x over previous
"""Causal dot-product attention (B=4, H=16, S=2048, D=128) on 8 TRN2 NeuronCores.

Sharding: batch*heads = 64 (b,h) pairs -> 8 heads per core (head parallel, no
communication). Each core runs a flash-attention-style kernel.

V2 design (ACT-engine-bound; exp() on the Scalar engine is the bottleneck):
  - Host pre-transposes Q,K per head to [D=128, S] in bf16 (halves DMA vs
    fp32r, same PE rate, enables FWL weight loads), and packs V per head as
    [kpos=128, kblock, D+1] bf16 with a ones column (PV matmul produces the
    softmax denominator for free).
  - st[k, q] blocks via bf16 matmuls. Diagonal handling: for q-tile i
    (256 q cols), full-width chunks j=0..2i (moving 256); the last chunk
    j=2i+1 is computed at HALF width (only q sub-block 1, moving 128) since
    its first sub-block is fully masked -- saves ~8% of exp elements and QK
    cycles vs the v1 kernel.
  - exp(scale*st) on the scalar engine in streaming groups of ~1024 PSUM
    columns -> bf16 pt in SBUF. Triangular mask multiplies on diagonal
    blocks alternate between DVE and GpSimd queues.
  - PV: out[q, 0:129] += pt_block.T @ V_aug accumulated in PSUM per q-tile
    ([128, 2, 129] = one bank). Deferred two groups so the in-order PE queue
    never head-of-line blocks on an in-flight exp.
  - Normalize batched per q-tile: one reciprocal [128,2] + one broadcast
    tensor_mul [128,2,128] -> bf16 out tile, one DMA per q-tile. Host
    upcasts bf16 -> f32.
  - Whole-head bulk DMAs (2 per tensor) + full next-head prefetch; warmup
    activation at t=0 forces the exp table load off the critical path.

No max-subtraction is needed: scores are ~N(0,1) after the 1/sqrt(128)
scale, so exp() stays in [e-6, e+6] comfortably inside bf16 range.
"""

import math
import sys
from contextlib import ExitStack

import numpy as np

for _p in ("/opt/trn_rl_repo", "/root/.axon_site/_ro/trn_rl_repo"):
    if _p not in sys.path:
        sys.path.append(_p)

import ml_dtypes

import concourse.bass as bass
import concourse.tile as tile
from concourse import bacc, mybir
from concourse.bass_utils import run_bass_kernel_spmd

F32 = mybir.dt.float32
BF16 = mybir.dt.bfloat16
AF = mybir.ActivationFunctionType

# Problem constants (hardcoded; kernel.py must be self-contained).
B, H, S, D = 4, 16, 2048, 128
P = 128
N_CORES = 8
NH = (B * H) // N_CORES  # heads per core = 8
SCALE = 1.0 / math.sqrt(128.0)  # D_MODEL = 128

QTW = 256   # q-tile width (2 sub-blocks of 128)
FILL = 1536  # st/pt group size in columns (3 PSUM banks)


def build_nc(nh=NH, s=S):
    nkb = s // P    # k-blocks per head = 16
    nqt = s // QTW  # q-tiles per head = 8

    nc = bacc.Bacc("TRN2", target_bir_lowering=False, debug=False,
                   enable_asserts=False)
    qt_d = nc.declare_dram_parameter("qt", [nh, P, s], BF16, isOutput=False).ap()
    kt_d = nc.declare_dram_parameter("kt", [nh, P, s], BF16, isOutput=False).ap()
    v_d = nc.declare_dram_parameter("v", [nh, P, nkb, D + 1], BF16,
                                    isOutput=False).ap()
    mask_d = nc.declare_dram_parameter("mask", [P, P], BF16, isOutput=False).ap()
    out_d = nc.declare_dram_parameter("out", [nh, s, D], BF16, isOutput=True).ap()

    with tile.TileContext(nc) as tc, ExitStack() as ctx:
        kt_pool = ctx.enter_context(tc.tile_pool(name="kt_pool", bufs=2))
        v_pool = ctx.enter_context(tc.tile_pool(name="v_pool", bufs=2))
        qt_pool = ctx.enter_context(tc.tile_pool(name="qt_pool", bufs=2))
        pt_pool = ctx.enter_context(tc.tile_pool(name="pt_pool", bufs=6))
        st_pool = ctx.enter_context(tc.tile_pool(name="st_pool", bufs=2,
                                                 space="PSUM"))
        acc_pool = ctx.enter_context(tc.tile_pool(name="acc_pool", bufs=2,
                                                  space="PSUM"))
        out_pool = ctx.enter_context(tc.tile_pool(name="out_pool", bufs=4))
        rl_pool = ctx.enter_context(tc.tile_pool(name="rl_pool", bufs=4))
        misc = ctx.enter_context(tc.tile_pool(name="misc", bufs=1))

        # Warmup activation: forces the exp table load at t=0, off the
        # critical path (the real first exp otherwise pays ~2.7us).
        warm = misc.tile([P, 8], F32)
        nc.vector.memset(warm[:], 0.0)
        nc.scalar.activation(warm[:], warm[:], AF.Exp, bias=0.0, scale=1.0)

        mask_t = misc.tile([P, P], BF16)
        nc.sync.dma_start(out=mask_t[:], in_=mask_d)

        # Streaming group state: st/pt tiles fill with chunks until FILL
        # columns, then one exp() drains them; PV consumption is deferred
        # two groups (lag) so the in-order PE queue never blocks ready QK
        # work behind a PV whose exp is still in flight.
        #
        # Half-width (128-col) diagonal chunks are deferred to the group's
        # tail ("halves"): full 256-col chunks then always start at
        # 256-aligned offsets and half chunks at 128-aligned offsets, so no
        # matmul output ever crosses a 512-float PSUM bank boundary (which
        # is illegal and corrupts nondeterministically on HW).
        state = {"st": None, "pt": None, "fill": 0, "entries": [],
                 "pending": [], "mask_tick": 0, "halves": []}

        def normalize(h, i, acc_t):
            rl = rl_pool.tile([P, 2, 1], F32, tag="rl", name="rl")
            nc.vector.reciprocal(rl[:], acc_t[:, :, 128:129])
            o_t = out_pool.tile([P, 2, D], BF16, tag="o", name="o_t")
            nc.vector.tensor_mul(o_t[:], acc_t[:, :, 0:128],
                                 rl[:, :, 0:1].broadcast_to([P, 2, D]))
            dst = out_d[h, i * QTW:(i + 1) * QTW, :].rearrange(
                "(si q) d -> q si d", si=2)
            nc.gpsimd.dma_start(out=dst, in_=o_t[:])

        def emit_pv(group):
            pt_t, entries = group
            for (pos, width, eh, i, j, acc_t, v_t) in entries:
                if width == QTW:
                    # full chunk: j in 0..2i; sI=0 masked iff j==2i
                    ps0 = pt_t[:, pos:pos + P]
                    if j == 2 * i:
                        nc.vector.tensor_mul(ps0, ps0, mask_t[:])
                    # One PSUM accumulation group per acc bank: start=True
                    # arms the whole 2KB zero region, so only the first
                    # matmul into the tile starts and only the last stops.
                    nc.tensor.matmul(acc_t[:, 0, :], lhsT=ps0, rhs=v_t[:, j],
                                     start=(j == 0), stop=False)
                    ps1 = pt_t[:, pos + P:pos + QTW]
                    nc.tensor.matmul(acc_t[:, 1, :], lhsT=ps1, rhs=v_t[:, j],
                                     start=False, stop=False)
                else:
                    # half chunk: j == 2i+1, only q sub-block 1, always masked
                    ps1 = pt_t[:, pos:pos + P]
                    nc.vector.tensor_mul(ps1, ps1, mask_t[:])
                    nc.tensor.matmul(acc_t[:, 1, :], lhsT=ps1, rhs=v_t[:, j],
                                     start=False, stop=True)
            for (pos, width, eh, i, j, acc_t, v_t) in entries:
                if width != QTW:
                    normalize(eh, i, acc_t)

        def emit_qk(kt_t, qt_t, v_t, h, i, j, acc_t, width, qoff):
            pos = state["fill"]
            nc.tensor.matmul(state["st"][:, pos:pos + width],
                             lhsT=kt_t[:, j * P:(j + 1) * P],
                             rhs=qt_t[:, qoff:qoff + width],
                             start=True, stop=True)
            state["entries"].append((pos, width, h, i, j, acc_t, v_t))
            state["fill"] += width

        def flush(final=False):
            # Emit deferred half chunks at the (128-aligned) tail first.
            if state["halves"] and state["st"] is None:
                state["st"] = st_pool.tile([P, FILL], F32, tag="st", name="st_t")
                state["pt"] = pt_pool.tile([P, FILL], BF16, tag="pt", name="pt_t")
            for half in state["halves"]:
                emit_qk(*half)
            state["halves"] = []
            pend = state["pending"]
            if state["fill"]:
                w = state["fill"]
                st_t, pt_t = state["st"], state["pt"]
                nc.scalar.activation(pt_t[:, :w], st_t[:, :w], AF.Exp,
                                     bias=0.0, scale=SCALE)
                pend.append((pt_t, state["entries"]))
            lag = 0 if final else 2
            while len(pend) > lag:
                emit_pv(pend.pop(0))
            state.update(st=None, pt=None, fill=0, entries=[], pending=pend)

        def add_chunk(kt_t, qt_t, v_t, h, i, j, acc_t, width, qoff):
            if width == P:
                # Defer to this group's tail so full chunks stay 256-aligned.
                state["halves"].append(
                    (kt_t, qt_t, v_t, h, i, j, acc_t, width, qoff))
                return
            if state["fill"] + width + P * len(state["halves"]) > FILL:
                flush()
            if state["fill"] == 0:
                state["st"] = st_pool.tile([P, FILL], F32, tag="st", name="st_t")
                state["pt"] = pt_pool.tile([P, FILL], BF16, tag="pt", name="pt_t")
            emit_qk(kt_t, qt_t, v_t, h, i, j, acc_t, width, qoff)
            if state["fill"] + P * len(state["halves"]) >= FILL:
                flush()

        def load_head(h, tiles):
            """Allocate head h's tiles and emit its loads (ordered so the
            first q-tile's needs land first)."""
            kt_t = kt_pool.tile([P, s], BF16, tag="kt", name="kt_t")
            qt_t = qt_pool.tile([P, s], BF16, tag="qt", name="qt_t")
            v_t = v_pool.tile([P, nkb, D + 1], BF16, tag="v", name="v_t")
            tiles[h] = (kt_t, qt_t, v_t)
            if h == 0:
                # Fine-grained first slices so the first q-tile's QK (and
                # with it the exp pipeline) starts as early as possible; kt
                # issues go on the (otherwise idle) gpsimd queue so the two
                # streams issue in parallel (DMA-capable queues: sync/scalar/gpsimd).
                nc.gpsimd.dma_start(out=kt_t[:, :256], in_=kt_d[h, :, :256])
                nc.sync.dma_start(out=qt_t[:, :256], in_=qt_d[h, :, :256])
                nc.gpsimd.dma_start(out=kt_t[:, 256:1024], in_=kt_d[h, :, 256:1024])
                nc.sync.dma_start(out=qt_t[:, 256:1024], in_=qt_d[h, :, 256:1024])
                nc.sync.dma_start(out=v_t[:, :4], in_=v_d[h, :, :4])
                nc.gpsimd.dma_start(out=kt_t[:, 1024:], in_=kt_d[h, :, 1024:])
                nc.sync.dma_start(out=qt_t[:, 1024:], in_=qt_d[h, :, 1024:])
                nc.sync.dma_start(out=v_t[:, 4:], in_=v_d[h, :, 4:])
            else:
                nc.sync.dma_start(out=kt_t[:, :1024], in_=kt_d[h, :, :1024])
                nc.sync.dma_start(out=qt_t[:, :1024], in_=qt_d[h, :, :1024])
                nc.sync.dma_start(out=v_t[:, :8], in_=v_d[h, :, :8])
                nc.sync.dma_start(out=kt_t[:, 1024:], in_=kt_d[h, :, 1024:])
                nc.sync.dma_start(out=qt_t[:, 1024:], in_=qt_d[h, :, 1024:])
                nc.sync.dma_start(out=v_t[:, 8:], in_=v_d[h, :, 8:])

        # Forced flush points: small groups at the very start (exp begins
        # before the bulk DMAs land) and at the very end (the post-last-exp
        # PV/normalize drain is tiny).
        force_tile = {(0, 0), (0, 1)}
        force_chunk = {(nh - 1, nqt - 1, 2 * nqt - 4), (nh - 1, nqt - 1, 2 * nqt - 3),
                       (nh - 1, nqt - 1, 2 * nqt - 2)}

        tiles = {}
        load_head(0, tiles)
        for h in range(nh):
            kt_t, qt_t, v_t = tiles[h]
            for i in range(nqt):
                if h + 1 < nh and i == 4:
                    load_head(h + 1, tiles)
                acc_t = acc_pool.tile([P, 2, 129], F32, tag="acc", name="acc_t")
                qoff = i * QTW
                for j in range(2 * i + 1):
                    add_chunk(kt_t, qt_t, v_t, h, i, j, acc_t, QTW, qoff)
                    if (h, i, j) in force_chunk:
                        flush()
                # last (odd-diagonal) chunk at half width: q sub-block 1 only
                add_chunk(kt_t, qt_t, v_t, h, i, 2 * i + 1, acc_t, P, qoff + P)
                if (h, i) in force_tile:
                    flush()
        flush(final=True)
    nc.compile()
    return nc


_NC = None


def _get_nc():
    global _NC
    if _NC is None:
        _NC = build_nc()
    return _NC


def prepare_in_maps(Q, K, V):
    """Shard + lay out full [B,H,S,D] inputs into per-core in_maps."""
    Qf = np.ascontiguousarray(np.asarray(Q, dtype=np.float32)).reshape(B * H, S, D)
    Kf = np.ascontiguousarray(np.asarray(K, dtype=np.float32)).reshape(B * H, S, D)
    Vf = np.ascontiguousarray(np.asarray(V, dtype=np.float32)).reshape(B * H, S, D)
    nkb = S // P
    # mask[k, q] = 1 iff q >= k (keep lower-triangular scores)
    mask = np.triu(np.ones((P, P), dtype=np.float32)).astype(ml_dtypes.bfloat16)
    in_maps = []
    for c in range(N_CORES):
        hs = slice(c * NH, (c + 1) * NH)
        qt = np.ascontiguousarray(
            Qf[hs].transpose(0, 2, 1).astype(ml_dtypes.bfloat16))  # [NH, D, S]
        kt = np.ascontiguousarray(
            Kf[hs].transpose(0, 2, 1).astype(ml_dtypes.bfloat16))  # [NH, D, S]
        # V: [NH, S, D] -> [NH, kblock, kpos, D] -> [NH, kpos, kblock, D]
        vv = Vf[hs].reshape(NH, nkb, P, D).transpose(0, 2, 1, 3)
        v_aug = np.ones((NH, P, nkb, D + 1), dtype=ml_dtypes.bfloat16)
        v_aug[..., :D] = vv.astype(ml_dtypes.bfloat16)
        in_maps.append({"qt": qt, "kt": kt, "v": v_aug, "mask": mask})
    return in_maps


def gather_out(results):
    out = np.concatenate([np.asarray(r["out"]).astype(np.float32)
                          for r in results], axis=0)  # [64, S, D]
    return out.reshape(B, H, S, D)


def kernel(Q, K, V):
    in_maps = prepare_in_maps(Q, K, V)
    nc = _get_nc()
    res = run_bass_kernel_spmd(nc, in_maps, core_ids=list(range(N_CORES)))
    return gather_out(res.results)


# revision 22
# speedup vs baseline: 1.2346x; 1.0137x over previous
"""Causal dot-product attention (B=4, H=16, S=2048, D=128) on 8 TRN2 NeuronCores.

Sharding: batch*heads = 64 (b,h) pairs -> 8 heads per core (head parallel, no
communication). Each core runs a flash-attention-style kernel.

V2 design (ACT-engine-bound; exp() on the Scalar engine is the bottleneck):
  - Host pre-transposes Q,K per head to [D=128, S] in bf16 (halves DMA vs
    fp32r, same PE rate, enables FWL weight loads), and packs V per head as
    [kpos=128, kblock, D+1] bf16 with a ones column (PV matmul produces the
    softmax denominator for free).
  - st[k, q] blocks via bf16 matmuls. Diagonal handling: for q-tile i
    (256 q cols), full-width chunks j=0..2i (moving 256); the last chunk
    j=2i+1 is computed at HALF width (only q sub-block 1, moving 128) since
    its first sub-block is fully masked -- saves ~8% of exp elements and QK
    cycles vs the v1 kernel.
  - exp(scale*st) on the scalar engine in streaming groups of ~1024 PSUM
    columns -> bf16 pt in SBUF. Triangular mask multiplies on diagonal
    blocks alternate between DVE and GpSimd queues.
  - PV: out[q, 0:129] += pt_block.T @ V_aug accumulated in PSUM per q-tile
    ([128, 2, 129] = one bank). Deferred two groups so the in-order PE queue
    never head-of-line blocks on an in-flight exp.
  - Normalize batched per q-tile: one reciprocal [128,2] + one broadcast
    tensor_mul [128,2,128] -> bf16 out tile, one DMA per q-tile. Host
    upcasts bf16 -> f32.
  - Whole-head bulk DMAs (2 per tensor) + full next-head prefetch; warmup
    activation at t=0 forces the exp table load off the critical path.

No max-subtraction is needed: scores are ~N(0,1) after the 1/sqrt(128)
scale, so exp() stays in [e-6, e+6] comfortably inside bf16 range.
"""

import math
import sys
from contextlib import ExitStack

import numpy as np

for _p in ("/opt/trn_rl_repo", "/root/.axon_site/_ro/trn_rl_repo"):
    if _p not in sys.path:
        sys.path.append(_p)

import ml_dtypes

import concourse.bass as bass
import concourse.tile as tile
from concourse import bacc, mybir
from concourse.bass_utils import run_bass_kernel_spmd

F32 = mybir.dt.float32
BF16 = mybir.dt.bfloat16
AF = mybir.ActivationFunctionType

# Problem constants (hardcoded; kernel.py must be self-contained).
B, H, S, D = 4, 16, 2048, 128
P = 128
N_CORES = 8
NH = (B * H) // N_CORES  # heads per core = 8
SCALE = 1.0 / math.sqrt(128.0)  # D_MODEL = 128

QTW = 256   # q-tile width (2 sub-blocks of 128)
FILL = 1536  # st/pt group size in columns (3 PSUM banks)


def build_nc(nh=NH, s=S):
    nkb = s // P    # k-blocks per head = 16
    nqt = s // QTW  # q-tiles per head = 8

    nc = bacc.Bacc("TRN2", target_bir_lowering=False, debug=False,
                   enable_asserts=False)
    qt_d = nc.declare_dram_parameter("qt", [nh, P, s], BF16, isOutput=False).ap()
    kt_d = nc.declare_dram_parameter("kt", [nh, P, s], BF16, isOutput=False).ap()
    v_d = nc.declare_dram_parameter("v", [nh, P, nkb, D + 1], BF16,
                                    isOutput=False).ap()
    mask_d = nc.declare_dram_parameter("mask", [P, P], BF16, isOutput=False).ap()
    out_d = nc.declare_dram_parameter("out", [nh, s, D], BF16, isOutput=True).ap()

    with tile.TileContext(nc) as tc, ExitStack() as ctx:
        kt_pool = ctx.enter_context(tc.tile_pool(name="kt_pool", bufs=2))
        v_pool = ctx.enter_context(tc.tile_pool(name="v_pool", bufs=2))
        qt_pool = ctx.enter_context(tc.tile_pool(name="qt_pool", bufs=2))
        pt_pool = ctx.enter_context(tc.tile_pool(name="pt_pool", bufs=10))
        st_pool = ctx.enter_context(tc.tile_pool(name="st_pool", bufs=2,
                                                 space="PSUM"))
        acc_pool = ctx.enter_context(tc.tile_pool(name="acc_pool", bufs=2,
                                                  space="PSUM"))
        out_pool = ctx.enter_context(tc.tile_pool(name="out_pool", bufs=6))
        rl_pool = ctx.enter_context(tc.tile_pool(name="rl_pool", bufs=6))
        misc = ctx.enter_context(tc.tile_pool(name="misc", bufs=1))

        # Warmup activation: forces the exp table load at t=0, off the
        # critical path (the real first exp otherwise pays ~2.7us).
        warm = misc.tile([P, 8], F32)
        nc.vector.memset(warm[:], 0.0)
        nc.scalar.activation(warm[:], warm[:], AF.Exp, bias=0.0, scale=1.0)

        # Mask DMA issued from the scalar queue: the sync queue's first slots
        # stay free for the critical first qt slices (mask isn't needed until
        # the first emit_pv, ~2 groups later).
        mask_t = misc.tile([P, P], BF16)
        nc.scalar.dma_start(out=mask_t[:], in_=mask_d)

        # Streaming group state: st/pt tiles fill with chunks until FILL
        # columns, then one exp() drains them; PV consumption is deferred
        # two groups (lag) so the in-order PE queue never blocks ready QK
        # work behind a PV whose exp is still in flight.
        #
        # Half-width (128-col) diagonal chunks are deferred to the group's
        # tail ("halves"): full 256-col chunks then always start at
        # 256-aligned offsets and half chunks at 128-aligned offsets, so no
        # matmul output ever crosses a 512-float PSUM bank boundary (which
        # is illegal and corrupts nondeterministically on HW).
        state = {"st": None, "pt": None, "fill": 0, "entries": [],
                 "pending": [], "mask_tick": 0, "halves": []}

        def normalize(h, i, acc_t):
            rl = rl_pool.tile([P, 2, 1], F32, tag="rl", name="rl")
            nc.vector.reciprocal(rl[:], acc_t[:, :, 128:129])
            o_t = out_pool.tile([P, 2, D], BF16, tag="o", name="o_t")
            dst = out_d[h, i * QTW:(i + 1) * QTW, :].rearrange(
                "(si q) d -> q si d", si=2)
            if (h, i) == (nh - 1, nqt - 1):
                # Final q-tile: split per sub-block so the first half's
                # store overlaps the second half's multiply (shorter tail).
                for sI in range(2):
                    nc.vector.tensor_mul(
                        o_t[:, sI:sI + 1, :], acc_t[:, sI:sI + 1, 0:128],
                        rl[:, sI:sI + 1, 0:1].broadcast_to([P, 1, D]))
                    nc.gpsimd.dma_start(out=dst[:, sI:sI + 1, :],
                                        in_=o_t[:, sI:sI + 1, :])
            else:
                nc.vector.tensor_mul(o_t[:], acc_t[:, :, 0:128],
                                     rl[:, :, 0:1].broadcast_to([P, 2, D]))
                nc.gpsimd.dma_start(out=dst, in_=o_t[:])

        def emit_pv(group):
            pt_t, entries = group
            for (pos, width, eh, i, j, acc_t, v_t) in entries:
                if width == QTW:
                    # full chunk: j in 0..2i; sI=0 masked iff j==2i
                    ps0 = pt_t[:, pos:pos + P]
                    if j == 2 * i:
                        nc.vector.tensor_mul(ps0, ps0, mask_t[:])
                    # One PSUM accumulation group per acc bank: start=True
                    # arms the whole 2KB zero region, so only the first
                    # matmul into the tile starts and only the last stops.
                    nc.tensor.matmul(acc_t[:, 0, :], lhsT=ps0, rhs=v_t[:, j],
                                     start=(j == 0), stop=False)
                    ps1 = pt_t[:, pos + P:pos + QTW]
                    nc.tensor.matmul(acc_t[:, 1, :], lhsT=ps1, rhs=v_t[:, j],
                                     start=False, stop=False)
                else:
                    # half chunk: j == 2i+1, only q sub-block 1, always masked
                    ps1 = pt_t[:, pos:pos + P]
                    nc.vector.tensor_mul(ps1, ps1, mask_t[:])
                    nc.tensor.matmul(acc_t[:, 1, :], lhsT=ps1, rhs=v_t[:, j],
                                     start=False, stop=True)
            for (pos, width, eh, i, j, acc_t, v_t) in entries:
                if width != QTW:
                    normalize(eh, i, acc_t)

        def emit_qk(kt_t, qt_t, v_t, h, i, j, acc_t, width, qoff):
            pos = state["fill"]
            nc.tensor.matmul(state["st"][:, pos:pos + width],
                             lhsT=kt_t[:, j * P:(j + 1) * P],
                             rhs=qt_t[:, qoff:qoff + width],
                             start=True, stop=True)
            state["entries"].append((pos, width, h, i, j, acc_t, v_t))
            state["fill"] += width

        def flush(final=False):
            # Emit deferred half chunks at the (128-aligned) tail first.
            if state["halves"] and state["st"] is None:
                state["st"] = st_pool.tile([P, FILL], F32, tag="st", name="st_t")
                state["pt"] = pt_pool.tile([P, FILL], BF16, tag="pt", name="pt_t")
            for half in state["halves"]:
                emit_qk(*half)
            state["halves"] = []
            pend = state["pending"]
            if state["fill"]:
                w = state["fill"]
                st_t, pt_t = state["st"], state["pt"]
                nc.scalar.activation(pt_t[:, :w], st_t[:, :w], AF.Exp,
                                     bias=0.0, scale=SCALE)
                pend.append((pt_t, state["entries"]))
            lag = 0 if final else 2
            while len(pend) > lag:
                emit_pv(pend.pop(0))
            state.update(st=None, pt=None, fill=0, entries=[], pending=pend)

        def add_chunk(kt_t, qt_t, v_t, h, i, j, acc_t, width, qoff):
            if width == P:
                # Defer to this group's tail so full chunks stay 256-aligned.
                state["halves"].append(
                    (kt_t, qt_t, v_t, h, i, j, acc_t, width, qoff))
                return
            if state["fill"] + width + P * len(state["halves"]) > FILL:
                flush()
            if state["fill"] == 0:
                state["st"] = st_pool.tile([P, FILL], F32, tag="st", name="st_t")
                state["pt"] = pt_pool.tile([P, FILL], BF16, tag="pt", name="pt_t")
            emit_qk(kt_t, qt_t, v_t, h, i, j, acc_t, width, qoff)
            if state["fill"] + P * len(state["halves"]) >= FILL:
                flush()

        def load_head(h, tiles):
            """Allocate head h's tiles and emit its loads (ordered so the
            first q-tile's needs land first)."""
            kt_t = kt_pool.tile([P, s], BF16, tag="kt", name="kt_t")
            qt_t = qt_pool.tile([P, s], BF16, tag="qt", name="qt_t")
            v_t = v_pool.tile([P, nkb, D + 1], BF16, tag="v", name="v_t")
            tiles[h] = (kt_t, qt_t, v_t)
            if h == 0:
                # Fine-grained first slices so the first q-tile's QK (and
                # with it the exp pipeline) starts as early as possible; kt
                # issues go on the (otherwise idle) gpsimd queue so the two
                # streams issue in parallel (DMA-capable queues: sync/scalar/gpsimd).
                nc.gpsimd.dma_start(out=kt_t[:, :256], in_=kt_d[h, :, :256])
                nc.sync.dma_start(out=qt_t[:, :256], in_=qt_d[h, :, :256])
                nc.gpsimd.dma_start(out=kt_t[:, 256:1024], in_=kt_d[h, :, 256:1024])
                nc.sync.dma_start(out=qt_t[:, 256:1024], in_=qt_d[h, :, 256:1024])
                nc.sync.dma_start(out=v_t[:, :4], in_=v_d[h, :, :4])
                nc.gpsimd.dma_start(out=kt_t[:, 1024:], in_=kt_d[h, :, 1024:])
                nc.sync.dma_start(out=qt_t[:, 1024:], in_=qt_d[h, :, 1024:])
                nc.sync.dma_start(out=v_t[:, 4:], in_=v_d[h, :, 4:])
            else:
                nc.sync.dma_start(out=kt_t[:, :1024], in_=kt_d[h, :, :1024])
                nc.sync.dma_start(out=qt_t[:, :1024], in_=qt_d[h, :, :1024])
                nc.sync.dma_start(out=v_t[:, :8], in_=v_d[h, :, :8])
                nc.sync.dma_start(out=kt_t[:, 1024:], in_=kt_d[h, :, 1024:])
                nc.sync.dma_start(out=qt_t[:, 1024:], in_=qt_d[h, :, 1024:])
                nc.sync.dma_start(out=v_t[:, 8:], in_=v_d[h, :, 8:])

        # Forced flush points: small groups at the very start (exp begins
        # before the bulk DMAs land) and at the very end (the post-last-exp
        # PV/normalize drain is tiny).
        force_tile = {(0, 0), (0, 1)}
        force_chunk = {(nh - 1, nqt - 1, 2 * nqt - 4), (nh - 1, nqt - 1, 2 * nqt - 3),
                       (nh - 1, nqt - 1, 2 * nqt - 2)}

        tiles = {}
        load_head(0, tiles)
        for h in range(nh):
            kt_t, qt_t, v_t = tiles[h]
            for i in range(nqt):
                if h + 1 < nh and i == 4:
                    load_head(h + 1, tiles)
                acc_t = acc_pool.tile([P, 2, 129], F32, tag="acc", name="acc_t")
                qoff = i * QTW
                for j in range(2 * i + 1):
                    add_chunk(kt_t, qt_t, v_t, h, i, j, acc_t, QTW, qoff)
                    if (h, i, j) in force_chunk:
                        flush()
                # last (odd-diagonal) chunk at half width: q sub-block 1 only
                add_chunk(kt_t, qt_t, v_t, h, i, 2 * i + 1, acc_t, P, qoff + P)
                if (h, i) in force_tile:
                    flush()
        flush(final=True)
    nc.compile()
    return nc


_NC = None


def _get_nc():
    global _NC
    if _NC is None:
        _NC = build_nc()
    return _NC


def prepare_in_maps(Q, K, V):
    """Shard + lay out full [B,H,S,D] inputs into per-core in_maps."""
    Qf = np.ascontiguousarray(np.asarray(Q, dtype=np.float32)).reshape(B * H, S, D)
    Kf = np.ascontiguousarray(np.asarray(K, dtype=np.float32)).reshape(B * H, S, D)
    Vf = np.ascontiguousarray(np.asarray(V, dtype=np.float32)).reshape(B * H, S, D)
    nkb = S // P
    # mask[k, q] = 1 iff q >= k (keep lower-triangular scores)
    mask = np.triu(np.ones((P, P), dtype=np.float32)).astype(ml_dtypes.bfloat16)
    in_maps = []
    for c in range(N_CORES):
        hs = slice(c * NH, (c + 1) * NH)
        qt = np.ascontiguousarray(
            Qf[hs].transpose(0, 2, 1).astype(ml_dtypes.bfloat16))  # [NH, D, S]
        kt = np.ascontiguousarray(
            Kf[hs].transpose(0, 2, 1).astype(ml_dtypes.bfloat16))  # [NH, D, S]
        # V: [NH, S, D] -> [NH, kblock, kpos, D] -> [NH, kpos, kblock, D]
        vv = Vf[hs].reshape(NH, nkb, P, D).transpose(0, 2, 1, 3)
        v_aug = np.ones((NH, P, nkb, D + 1), dtype=ml_dtypes.bfloat16)
        v_aug[..., :D] = vv.astype(ml_dtypes.bfloat16)
        in_maps.append({"qt": qt, "kt": kt, "v": v_aug, "mask": mask})
    return in_maps


def gather_out(results):
    out = np.concatenate([np.asarray(r["out"]).astype(np.float32)
                          for r in results], axis=0)  # [64, S, D]
    return out.reshape(B, H, S, D)


def kernel(Q, K, V):
    in_maps = prepare_in_maps(Q, K, V)
    nc = _get_nc()
    res = run_bass_kernel_spmd(nc, in_maps, core_ids=list(range(N_CORES)))
    return gather_out(res.results)


# revision 25
# speedup vs baseline: 1.2417x; 1.0057x over previous
"""Causal dot-product attention (B=4, H=16, S=2048, D=128) on 8 TRN2 NeuronCores.

Sharding: batch*heads = 64 (b,h) pairs -> 8 heads per core (head parallel, no
communication). Each core runs a flash-attention-style kernel.

V2 design (ACT-engine-bound; exp() on the Scalar engine is the bottleneck):
  - Host pre-transposes Q,K per head to [D=128, S] in bf16 (halves DMA vs
    fp32r, same PE rate, enables FWL weight loads), and packs V per head as
    [kpos=128, kblock, D+1] bf16 with a ones column (PV matmul produces the
    softmax denominator for free).
  - st[k, q] blocks via bf16 matmuls. Diagonal handling: for q-tile i
    (256 q cols), full-width chunks j=0..2i (moving 256); the last chunk
    j=2i+1 is computed at HALF width (only q sub-block 1, moving 128) since
    its first sub-block is fully masked -- saves ~8% of exp elements and QK
    cycles vs the v1 kernel.
  - exp(scale*st) on the scalar engine in streaming groups of ~1024 PSUM
    columns -> bf16 pt in SBUF. Triangular mask multiplies on diagonal
    blocks alternate between DVE and GpSimd queues.
  - PV: out[q, 0:129] += pt_block.T @ V_aug accumulated in PSUM per q-tile
    ([128, 2, 129] = one bank). Deferred two groups so the in-order PE queue
    never head-of-line blocks on an in-flight exp.
  - Normalize batched per q-tile: one reciprocal [128,2] + one broadcast
    tensor_mul [128,2,128] -> bf16 out tile, one DMA per q-tile. Host
    upcasts bf16 -> f32.
  - Whole-head bulk DMAs (2 per tensor) + full next-head prefetch; warmup
    activation at t=0 forces the exp table load off the critical path.

No max-subtraction is needed: scores are ~N(0,1) after the 1/sqrt(128)
scale, so exp() stays in [e-6, e+6] comfortably inside bf16 range.
"""

import math
import sys
from contextlib import ExitStack

import numpy as np

for _p in ("/opt/trn_rl_repo", "/root/.axon_site/_ro/trn_rl_repo"):
    if _p not in sys.path:
        sys.path.append(_p)

import ml_dtypes

import concourse.bass as bass
import concourse.tile as tile
from concourse import bacc, mybir
from concourse.bass_utils import run_bass_kernel_spmd

F32 = mybir.dt.float32
BF16 = mybir.dt.bfloat16
AF = mybir.ActivationFunctionType

# Problem constants (hardcoded; kernel.py must be self-contained).
B, H, S, D = 4, 16, 2048, 128
P = 128
N_CORES = 8
NH = (B * H) // N_CORES  # heads per core = 8
SCALE = 1.0 / math.sqrt(128.0)  # D_MODEL = 128

QTW = 256   # q-tile width (2 sub-blocks of 128)
FILL = 1536  # st/pt group size in columns (3 PSUM banks)


def build_nc(nh=NH, s=S):
    nkb = s // P    # k-blocks per head = 16
    nqt = s // QTW  # q-tiles per head = 8

    nc = bacc.Bacc("TRN2", target_bir_lowering=False, debug=False,
                   enable_asserts=False)
    qt_d = nc.declare_dram_parameter("qt", [nh, P, s], BF16, isOutput=False).ap()
    kt_d = nc.declare_dram_parameter("kt", [nh, P, s], BF16, isOutput=False).ap()
    v_d = nc.declare_dram_parameter("v", [nh, P, nkb, D + 1], BF16,
                                    isOutput=False).ap()
    mask_d = nc.declare_dram_parameter("mask", [P, P], BF16, isOutput=False).ap()
    out_d = nc.declare_dram_parameter("out", [nh, s, D], BF16, isOutput=True).ap()

    with tile.TileContext(nc) as tc, ExitStack() as ctx:
        kt_pool = ctx.enter_context(tc.tile_pool(name="kt_pool", bufs=2))
        v_pool = ctx.enter_context(tc.tile_pool(name="v_pool", bufs=2))
        qt_pool = ctx.enter_context(tc.tile_pool(name="qt_pool", bufs=2))
        pt_pool = ctx.enter_context(tc.tile_pool(name="pt_pool", bufs=10))
        st_pool = ctx.enter_context(tc.tile_pool(name="st_pool", bufs=2,
                                                 space="PSUM"))
        acc_pool = ctx.enter_context(tc.tile_pool(name="acc_pool", bufs=2,
                                                  space="PSUM"))
        out_pool = ctx.enter_context(tc.tile_pool(name="out_pool", bufs=6))
        rl_pool = ctx.enter_context(tc.tile_pool(name="rl_pool", bufs=6))
        misc = ctx.enter_context(tc.tile_pool(name="misc", bufs=1))

        # Warmup activation: forces the exp table load at t=0, off the
        # critical path (the real first exp otherwise pays ~2.7us).
        warm = misc.tile([P, 8], F32)
        nc.vector.memset(warm[:], 0.0)
        nc.scalar.activation(warm[:], warm[:], AF.Exp, bias=0.0, scale=1.0)

        # Mask DMA issued from the scalar queue: the sync queue's first slots
        # stay free for the critical first qt slices (mask isn't needed until
        # the first emit_pv, ~2 groups later).
        mask_t = misc.tile([P, P], BF16)
        nc.scalar.dma_start(out=mask_t[:], in_=mask_d)

        # Streaming group state: st/pt tiles fill with chunks until FILL
        # columns, then one exp() drains them; PV consumption is deferred
        # two groups (lag) so the in-order PE queue never blocks ready QK
        # work behind a PV whose exp is still in flight.
        #
        # Half-width (128-col) diagonal chunks are deferred to the group's
        # tail ("halves"): full 256-col chunks then always start at
        # 256-aligned offsets and half chunks at 128-aligned offsets, so no
        # matmul output ever crosses a 512-float PSUM bank boundary (which
        # is illegal and corrupts nondeterministically on HW).
        state = {"st": None, "pt": None, "fill": 0, "entries": [],
                 "pending": [], "mask_tick": 0, "halves": []}

        def normalize(h, i, acc_t):
            rl = rl_pool.tile([P, 2, 1], F32, tag="rl", name="rl")
            nc.vector.reciprocal(rl[:], acc_t[:, :, 128:129])
            o_t = out_pool.tile([P, 2, D], BF16, tag="o", name="o_t")
            dst = out_d[h, i * QTW:(i + 1) * QTW, :].rearrange(
                "(si q) d -> q si d", si=2)
            nc.vector.tensor_mul(o_t[:], acc_t[:, :, 0:128],
                                 rl[:, :, 0:1].broadcast_to([P, 2, D]))
            nc.gpsimd.dma_start(out=dst, in_=o_t[:])

        def emit_pv(group):
            pt_t, entries = group
            for (pos, width, eh, i, j, acc_t, v_t) in entries:
                if width == QTW:
                    # full chunk: j in 0..2i; sI=0 masked iff j==2i
                    ps0 = pt_t[:, pos:pos + P]
                    if j == 2 * i:
                        nc.vector.tensor_mul(ps0, ps0, mask_t[:])
                    # One PSUM accumulation group per acc bank: start=True
                    # arms the whole 2KB zero region, so only the first
                    # matmul into the tile starts and only the last stops.
                    nc.tensor.matmul(acc_t[:, 0, :], lhsT=ps0, rhs=v_t[:, j],
                                     start=(j == 0), stop=False)
                    ps1 = pt_t[:, pos + P:pos + QTW]
                    nc.tensor.matmul(acc_t[:, 1, :], lhsT=ps1, rhs=v_t[:, j],
                                     start=False, stop=False)
                else:
                    # half chunk: j == 2i+1, only q sub-block 1, always masked
                    ps1 = pt_t[:, pos:pos + P]
                    nc.vector.tensor_mul(ps1, ps1, mask_t[:])
                    nc.tensor.matmul(acc_t[:, 1, :], lhsT=ps1, rhs=v_t[:, j],
                                     start=False, stop=True)
            for (pos, width, eh, i, j, acc_t, v_t) in entries:
                if width != QTW:
                    normalize(eh, i, acc_t)

        def emit_qk(kt_t, qt_t, v_t, h, i, j, acc_t, width, qoff):
            pos = state["fill"]
            nc.tensor.matmul(state["st"][:, pos:pos + width],
                             lhsT=kt_t[:, j * P:(j + 1) * P],
                             rhs=qt_t[:, qoff:qoff + width],
                             start=True, stop=True)
            state["entries"].append((pos, width, h, i, j, acc_t, v_t))
            state["fill"] += width

        def flush(final=False):
            # Emit deferred half chunks at the (128-aligned) tail first.
            if state["halves"] and state["st"] is None:
                state["st"] = st_pool.tile([P, FILL], F32, tag="st", name="st_t")
                state["pt"] = pt_pool.tile([P, FILL], BF16, tag="pt", name="pt_t")
            for half in state["halves"]:
                emit_qk(*half)
            state["halves"] = []
            pend = state["pending"]
            if state["fill"]:
                w = state["fill"]
                st_t, pt_t = state["st"], state["pt"]
                nc.scalar.activation(pt_t[:, :w], st_t[:, :w], AF.Exp,
                                     bias=0.0, scale=SCALE)
                pend.append((pt_t, state["entries"]))
            lag = 0 if final else 2
            while len(pend) > lag:
                emit_pv(pend.pop(0))
            state.update(st=None, pt=None, fill=0, entries=[], pending=pend)

        def add_chunk(kt_t, qt_t, v_t, h, i, j, acc_t, width, qoff):
            if width == P:
                # Defer to this group's tail so full chunks stay 256-aligned.
                state["halves"].append(
                    (kt_t, qt_t, v_t, h, i, j, acc_t, width, qoff))
                return
            if state["fill"] + width + P * len(state["halves"]) > FILL:
                flush()
            if state["fill"] == 0:
                state["st"] = st_pool.tile([P, FILL], F32, tag="st", name="st_t")
                state["pt"] = pt_pool.tile([P, FILL], BF16, tag="pt", name="pt_t")
            emit_qk(kt_t, qt_t, v_t, h, i, j, acc_t, width, qoff)
            if state["fill"] + P * len(state["halves"]) >= FILL:
                flush()

        def load_head(h, tiles):
            """Allocate head h's tiles and emit its loads (ordered so the
            first q-tile's needs land first)."""
            kt_t = kt_pool.tile([P, s], BF16, tag="kt", name="kt_t")
            qt_t = qt_pool.tile([P, s], BF16, tag="qt", name="qt_t")
            v_t = v_pool.tile([P, nkb, D + 1], BF16, tag="v", name="v_t")
            tiles[h] = (kt_t, qt_t, v_t)
            if h == 0:
                # Fine-grained first slices so the first q-tile's QK (and
                # with it the exp pipeline) starts as early as possible; kt
                # issues go on the (otherwise idle) gpsimd queue so the two
                # streams issue in parallel (DMA-capable queues: sync/scalar/gpsimd).
                nc.gpsimd.dma_start(out=kt_t[:, :256], in_=kt_d[h, :, :256])
                nc.sync.dma_start(out=qt_t[:, :256], in_=qt_d[h, :, :256])
                nc.gpsimd.dma_start(out=kt_t[:, 256:512], in_=kt_d[h, :, 256:512])
                nc.sync.dma_start(out=qt_t[:, 256:512], in_=qt_d[h, :, 256:512])
                nc.gpsimd.dma_start(out=kt_t[:, 512:1024], in_=kt_d[h, :, 512:1024])
                nc.sync.dma_start(out=qt_t[:, 512:1024], in_=qt_d[h, :, 512:1024])
                nc.sync.dma_start(out=v_t[:, :4], in_=v_d[h, :, :4])
                nc.gpsimd.dma_start(out=kt_t[:, 1024:], in_=kt_d[h, :, 1024:])
                nc.sync.dma_start(out=qt_t[:, 1024:], in_=qt_d[h, :, 1024:])
                nc.sync.dma_start(out=v_t[:, 4:], in_=v_d[h, :, 4:])
            else:
                nc.sync.dma_start(out=kt_t[:, :1024], in_=kt_d[h, :, :1024])
                nc.sync.dma_start(out=qt_t[:, :1024], in_=qt_d[h, :, :1024])
                nc.sync.dma_start(out=v_t[:, :8], in_=v_d[h, :, :8])
                nc.sync.dma_start(out=kt_t[:, 1024:], in_=kt_d[h, :, 1024:])
                nc.sync.dma_start(out=qt_t[:, 1024:], in_=qt_d[h, :, 1024:])
                nc.sync.dma_start(out=v_t[:, 8:], in_=v_d[h, :, 8:])

        # Forced flush points: small groups at the very start (exp begins
        # before the bulk DMAs land) and at the very end (the post-last-exp
        # PV/normalize drain is tiny).
        force_tile = {(0, 0), (0, 1)}
        force_chunk = {(0, 2, 2),
                       (nh - 1, nqt - 1, 2 * nqt - 4), (nh - 1, nqt - 1, 2 * nqt - 3),
                       (nh - 1, nqt - 1, 2 * nqt - 2)}

        tiles = {}
        load_head(0, tiles)
        for h in range(nh):
            kt_t, qt_t, v_t = tiles[h]
            for i in range(nqt):
                if h + 1 < nh and i == 4:
                    load_head(h + 1, tiles)
                acc_t = acc_pool.tile([P, 2, 129], F32, tag="acc", name="acc_t")
                qoff = i * QTW
                for j in range(2 * i + 1):
                    add_chunk(kt_t, qt_t, v_t, h, i, j, acc_t, QTW, qoff)
                    if (h, i, j) in force_chunk:
                        flush()
                # last (odd-diagonal) chunk at half width: q sub-block 1 only
                add_chunk(kt_t, qt_t, v_t, h, i, 2 * i + 1, acc_t, P, qoff + P)
                if (h, i) in force_tile:
                    flush()
        flush(final=True)
    nc.compile()
    return nc


_NC = None


def _get_nc():
    global _NC
    if _NC is None:
        _NC = build_nc()
    return _NC


def prepare_in_maps(Q, K, V):
    """Shard + lay out full [B,H,S,D] inputs into per-core in_maps."""
    Qf = np.ascontiguousarray(np.asarray(Q, dtype=np.float32)).reshape(B * H, S, D)
    Kf = np.ascontiguousarray(np.asarray(K, dtype=np.float32)).reshape(B * H, S, D)
    Vf = np.ascontiguousarray(np.asarray(V, dtype=np.float32)).reshape(B * H, S, D)
    nkb = S // P
    # mask[k, q] = 1 iff q >= k (keep lower-triangular scores)
    mask = np.triu(np.ones((P, P), dtype=np.float32)).astype(ml_dtypes.bfloat16)
    in_maps = []
    for c in range(N_CORES):
        hs = slice(c * NH, (c + 1) * NH)
        qt = np.ascontiguousarray(
            Qf[hs].transpose(0, 2, 1).astype(ml_dtypes.bfloat16))  # [NH, D, S]
        kt = np.ascontiguousarray(
            Kf[hs].transpose(0, 2, 1).astype(ml_dtypes.bfloat16))  # [NH, D, S]
        # V: [NH, S, D] -> [NH, kblock, kpos, D] -> [NH, kpos, kblock, D]
        vv = Vf[hs].reshape(NH, nkb, P, D).transpose(0, 2, 1, 3)
        v_aug = np.ones((NH, P, nkb, D + 1), dtype=ml_dtypes.bfloat16)
        v_aug[..., :D] = vv.astype(ml_dtypes.bfloat16)
        in_maps.append({"qt": qt, "kt": kt, "v": v_aug, "mask": mask})
    return in_maps


def gather_out(results):
    out = np.concatenate([np.asarray(r["out"]).astype(np.float32)
                          for r in results], axis=0)  # [64, S, D]
    return out.reshape(B, H, S, D)


def kernel(Q, K, V):
    in_maps = prepare_in_maps(Q, K, V)
    nc = _get_nc()
    res = run_bass_kernel_spmd(nc, in_maps, core_ids=list(range(N_CORES)))
    return gather_out(res.results)
